# revision 1
# baseline (speedup 1.0000x reference)
"""Cross-attention kernel for Trainium2, data-parallel over (batch, query-half)
across 8 NeuronCores.

Problem (per batch element b, with C=512 channels, N=64*64=4096 positions):
    q = Wq @ xt[b] + bq          [64, N]
    k = Wk @ xs[b] + bk          [64, N]
    v = Wv @ xs[b] + bv          [512, N]
    attn = softmax_j(q^T k)      [N, N]   (softmax over keys j)
    out = v @ attn^T             [512, N]
    y = gamma * out + xs[b]

Sharding: 8 cores = 4 batches x 2 query-halves. Each core holds full xs[b]
(keys/values are over all N positions) and its half of xt[b] (2048 queries);
weights are replicated. No collectives needed.

Per-core dataflow (all matmuls bf16 with fp32 PSUM accumulation, softmax
statistics and the residual epilogue in fp32):
  - Q as 4 group tiles [64, 512], K as 8 block tiles [64, 512], V^T as 32
    tiles [128, 512]
    (V^T[j, c] = sum_ch xs[ch, j] WvT[ch, c] comes out directly in the layout
    the attention matmul needs, since xs is already [ch, j]).
  - energy^T tiles [128 j, 512 i] = K_j^T-contracted matmul, exp on the
    scalar engine straight out of PSUM. No max-subtraction: energies here are
    inner products of 64-dim ~N(0,1) vectors (std ~8, max |e| ~46), and fp32
    exp is exact-range-safe up to 88.
  - out^T[i, c] accumulates over j in PSUM; the softmax denominator rides
    along as an n=1 matmul against a ones-vector reusing the same stationary
    operand. Normalization multiplies by reciprocal(sum) * gamma per query.
  - Output stays in [query, channel] layout: the residual input is passed
    pre-transposed (+gamma*bv folded in) and the host transposes the returned
    tensor, so no on-chip transposes are needed.

Queries run in 4 groups of 512; exp tiles alternate between two buffer sets so
adjacent groups overlap. Measured ~245 us on hardware (8 cores, SPMD).
"""

import numpy as np
import ml_dtypes

B, C, W, H = 4, 512, 64, 64
N = W * H            # 4096 keys per batch element
DQK = 64
NQ = N // 2          # queries per core
NCHUNK = C // 128    # 4 channel chunks
NJ = N // 128        # 32 key tiles
NGROUP = 4           # query groups per core
GQ = NQ // NGROUP    # 512 queries per group
NIT = GQ // 128      # 4 query tiles per group
NBLK = N // 512      # 8 key blocks of 512 for the K/V build
N_CORES = 8

_F32 = np.float32
_BF16 = ml_dtypes.bfloat16


def _split_multi_waits(nc, max_waits=1):
    """The walrus in this container rejects instructions carrying more than
    `max_waits` semaphore waits ("Too many sync wait commands" in
    setupSyncWait). Engines dispatch in order, so extra waits can be peeled
    onto NoOps inserted immediately before the instruction on the same
    engine without changing semantics."""
    from concourse import mybir

    for f in nc.m.functions:
        for bb in f.blocks:
            new_insts = []
            changed = False
            for inst in bb.instructions:
                si = inst.sync_info
                if si is not None and si.on_wait and len(si.on_wait) > max_waits:
                    waits = list(si.on_wait)
                    extra, keep = waits[:-max_waits], waits[-max_waits:]
                    for k in range(0, len(extra), max_waits):
                        nop = mybir.InstNoOp(
                            name=f"{inst.name}-ws{k}",
                            sync_info=mybir.SyncInfo(
                                on_wait=extra[k : k + max_waits], on_update=[]
                            ),
                        )
                        nop.engine = inst.engine
                        new_insts.append(nop)
                    inst.sync_info = mybir.SyncInfo(
                        on_wait=keep, on_update=list(si.on_update)
                    )
                    changed = True
                new_insts.append(inst)
            if changed:
                bb.instructions = new_insts


def build_program():
    import concourse.bass as bass
    import concourse.tile as tile
    from concourse import mybir
    from concourse.masks import make_identity

    f32 = mybir.dt.float32
    bf16 = mybir.dt.bfloat16
    Alu = mybir.AluOpType
    Act = mybir.ActivationFunctionType

    nc = bass.Bass("TRN2", target_bir_lowering=False, debug=False, num_devices=1)

    xs = nc.dram_tensor("xs", [C, N], f32, kind="ExternalInput").ap()
    xt = nc.dram_tensor("xt", [C, NQ], f32, kind="ExternalInput").ap()
    # x_s^T (this core's query half) + gamma*bv, for the residual epilogue
    xres = nc.dram_tensor("xrt", [NQ, C], f32, kind="ExternalInput").ap()
    wq = nc.dram_tensor("wq", [NCHUNK, 128, DQK], bf16, kind="ExternalInput").ap()
    wk = nc.dram_tensor("wk", [NCHUNK, 128, DQK], bf16, kind="ExternalInput").ap()
    wv = nc.dram_tensor("wv", [NCHUNK, 128, C], bf16, kind="ExternalInput").ap()
    bq = nc.dram_tensor("bq", [DQK, 1], f32, kind="ExternalInput").ap()
    bk = nc.dram_tensor("bk", [DQK, 1], f32, kind="ExternalInput").ap()
    gm = nc.dram_tensor("gm", [128, 1], f32, kind="ExternalInput").ap()
    out = nc.dram_tensor("outT", [NQ, C], f32, kind="ExternalOutput").ap()

    # [ (chunk, p) , n ] views of the fp32 activations
    xsv = xs.rearrange("(q p) n -> p q n", p=128)
    xtv = xt.rearrange("(q p) n -> p q n", p=128)
    # residual + out are kept transposed ([query, channel]); blocks of 128 rows
    xrv = xres.rearrange("(q p) c -> p q c", p=128)
    outv = out.rearrange("(q p) c -> p q c", p=128)

    with tile.TileContext(nc) as tc:
        with (
            tc.tile_pool(name="consts", bufs=1) as cpool,
            tc.tile_pool(name="acts", bufs=3) as apool,
            tc.tile_pool(name="qsb", bufs=1) as qpool,
            tc.tile_pool(name="ksb", bufs=1) as kpool,
            tc.tile_pool(name="vtsb", bufs=1) as vpool,
            tc.tile_pool(name="esb", bufs=1) as epool,
            tc.tile_pool(name="osb", bufs=1) as opool,
            tc.tile_pool(name="small", bufs=2) as spool,
            tc.tile_pool(name="epi", bufs=4) as fpool,
            tc.tile_pool(name="ps_misc", bufs=1, space="PSUM") as ps_misc,
            tc.tile_pool(name="ps_vt", bufs=1, space="PSUM") as ps_vt,
            tc.tile_pool(name="ps_e", bufs=2, space="PSUM") as ps_e,
            tc.tile_pool(name="ps_av", bufs=2, space="PSUM") as ps_av,
            tc.tile_pool(name="ps_sum", bufs=2, space="PSUM") as ps_sum,
        ):
            # ---- constants / weights ----
            ones = cpool.tile([128, 1], bf16, tag="ones")
            nc.vector.memset(ones[:, :], 1.0)

            wq_sb = cpool.tile([128, NCHUNK, DQK], bf16, tag="wq")
            nc.sync.dma_start(wq_sb[:, :, :], wq.rearrange("q p d -> p q d"))
            wk_sb = cpool.tile([128, NCHUNK, DQK], bf16, tag="wk")
            nc.sync.dma_start(wk_sb[:, :, :], wk.rearrange("q p d -> p q d"))
            wv_sb = cpool.tile([128, NCHUNK, C], bf16, tag="wv")
            nc.sync.dma_start(wv_sb[:, :, :], wv.rearrange("q p d -> p q d"))
            bq_sb = cpool.tile([DQK, 1], f32, tag="bq")
            nc.sync.dma_start(bq_sb[:, :], bq[:, :])
            bk_sb = cpool.tile([DQK, 1], f32, tag="bk")
            nc.sync.dma_start(bk_sb[:, :], bk[:, :])
            gm_sb = cpool.tile([128, 1], f32, tag="gm")
            nc.sync.dma_start(gm_sb[:, :], gm[:, :])

            # ---- Q [64, 512] per group, K [64, 512] per key block, V^T tiles
            # [128, 512] per key tile. Built in 512-column blocks (1 MB DMAs,
            # n=512 matmuls) with the Q build interleaved into the key loop so
            # the PE stream stays dense from the start.
            q_g = [
                qpool.tile([DQK, GQ], bf16, tag=f"q{g}", name=f"q{g}")
                for g in range(NGROUP)
            ]
            k_t = [
                kpool.tile([DQK, 512], bf16, tag=f"k{jq}", name=f"k{jq}")
                for jq in range(NBLK)
            ]
            vt_t = []
            for jq in range(NBLK):
                bsl = slice(jq * 512, (jq + 1) * 512)
                xsf = apool.tile([128, NCHUNK, 512], f32, tag="xsf")
                for qc in range(NCHUNK):
                    nc.sync.dma_start(xsf[:, qc, :], xsv[:, qc, bsl])
                xsb = apool.tile([128, NCHUNK, 512], bf16, tag="xsb")
                for qc in range(NCHUNK):
                    nc.vector.tensor_copy(xsb[:, qc, :], xsf[:, qc, :])

                for jt in range(4):
                    vt_ps = ps_vt.tile([128, C], f32, tag="vtp")
                    for qc in range(NCHUNK):
                        nc.tensor.matmul(
                            vt_ps[:, :],
                            xsb[:, qc, jt * 128 : (jt + 1) * 128],
                            wv_sb[:, qc, :],
                            start=(qc == 0),
                            stop=(qc == NCHUNK - 1),
                        )
                    j = jq * 4 + jt
                    vt_j = vpool.tile([128, C], bf16, tag=f"vt{j}", name=f"vt{j}")
                    nc.vector.tensor_copy(vt_j[:, :], vt_ps[:, :])
                    vt_t.append(vt_j)

                k_ps = ps_misc.tile([DQK, 512], f32, tag="misc")
                for qc in range(NCHUNK):
                    nc.tensor.matmul(
                        k_ps[:, :],
                        wk_sb[:, qc, :],
                        xsb[:, qc, :],
                        start=(qc == 0),
                        stop=(qc == NCHUNK - 1),
                    )
                nc.vector.tensor_scalar(
                    k_t[jq][:, :], k_ps[:, :], bk_sb[:, :], None, Alu.add
                )

                if jq < NGROUP:
                    g = jq
                    xtf = apool.tile([128, NCHUNK, 512], f32, tag="xtf")
                    for qc in range(NCHUNK):
                        nc.sync.dma_start(
                            xtf[:, qc, :], xtv[:, qc, g * GQ : (g + 1) * GQ]
                        )
                    xtb = apool.tile([128, NCHUNK, 512], bf16, tag="xtb")
                    for qc in range(NCHUNK):
                        nc.vector.tensor_copy(xtb[:, qc, :], xtf[:, qc, :])
                    q_ps = ps_misc.tile([DQK, 512], f32, tag="misc")
                    for qc in range(NCHUNK):
                        nc.tensor.matmul(
                            q_ps[:, :],
                            wq_sb[:, qc, :],
                            xtb[:, qc, :],
                            start=(qc == 0),
                            stop=(qc == NCHUNK - 1),
                        )
                    nc.vector.tensor_scalar(
                        q_g[g][:, :], q_ps[:, :], bq_sb[:, :], None, Alu.add
                    )

            # ---- attention, one query group at a time; exp tiles alternate
            # between two buffer sets so group g+1's energies/exps fill while
            # group g's AV matmuls are still consuming the other set ----
            for g in range(NGROUP):
                e_t = []
                for j in range(NJ):
                    e_ps = ps_e.tile([128, GQ], f32, tag="eps")
                    nc.tensor.matmul(
                        e_ps[:, :],
                        k_t[j // 4][:, (j % 4) * 128 : (j % 4 + 1) * 128],
                        q_g[g][:, :],
                        start=True,
                        stop=True,
                    )
                    e_j = epool.tile(
                        [128, GQ], bf16, tag=f"e{g % 2}_{j}", name=f"e{g}_{j}"
                    )
                    nc.scalar.activation(e_j[:, :], e_ps[:, :], Act.Exp)
                    e_t.append(e_j)

                for it in range(NIT):
                    av_ps = ps_av.tile([128, C], f32, tag="av")
                    s_ps = ps_sum.tile([128, 1], f32, tag="sm")
                    isl = slice(it * 128, (it + 1) * 128)
                    for j in range(NJ):
                        nc.tensor.matmul(
                            av_ps[:, :],
                            e_t[j][:, isl],
                            vt_t[j][:, :],
                            start=(j == 0),
                            stop=(j == NJ - 1),
                        )
                        nc.tensor.matmul(
                            s_ps[:, :],
                            e_t[j][:, isl],
                            ones[:, :],
                            start=(j == 0),
                            stop=(j == NJ - 1),
                        )
                    recip = spool.tile([128, 1], f32, tag="rc")
                    nc.vector.reciprocal(recip[:, :], s_ps[:, :])
                    # normalize, scale by gamma, keep [query, channel] layout;
                    # two half-width pieces pipeline DVE with the out DMA
                    blk = g * NIT + it
                    xr = fpool.tile([128, C], f32, tag="xr")
                    nc.sync.dma_start(xr[:, :], xrv[:, blk, :])
                    for hh in range(2):
                        csl = slice(hh * (C // 2), (hh + 1) * (C // 2))
                        t_o = opool.tile([128, C // 2], f32, tag="to")
                        nc.vector.tensor_scalar(
                            t_o[:, :],
                            av_ps[:, csl],
                            recip[:, :],
                            gm_sb[:, :],
                            Alu.mult,
                            Alu.mult,
                        )
                        of = fpool.tile([128, C // 2], f32, tag="of")
                        nc.vector.tensor_tensor(
                            of[:, :], t_o[:, :], xr[:, csl], Alu.add
                        )
                        nc.sync.dma_start(outv[:, blk, csl], of[:, :])

    _split_multi_waits(nc)
    return nc


_PROGRAM = None


def _get_program():
    global _PROGRAM
    if _PROGRAM is None:
        _PROGRAM = build_program()
    return _PROGRAM


def make_in_maps(x_s, x_t, Wq, bq, Wk, bk, Wv, bv, gamma):
    x_s = np.asarray(x_s, dtype=_F32)
    x_t = np.asarray(x_t, dtype=_F32)
    Wq = np.asarray(Wq, dtype=_F32)
    Wk = np.asarray(Wk, dtype=_F32)
    Wv = np.asarray(Wv, dtype=_F32)
    bq = np.asarray(bq, dtype=_F32)
    bk = np.asarray(bk, dtype=_F32)
    bv = np.asarray(bv, dtype=_F32)
    gamma = np.asarray(gamma, dtype=_F32)

    xs_full = x_s.reshape(B, C, N)
    xt_full = x_t.reshape(B, C, N)

    # host-side layout prep: pre-transposed bf16 weights, chunked for SBUF
    wq_h = np.ascontiguousarray(Wq.T.reshape(NCHUNK, 128, DQK)).astype(_BF16)
    wk_h = np.ascontiguousarray(Wk.T.reshape(NCHUNK, 128, DQK)).astype(_BF16)
    wv_h = np.ascontiguousarray(Wv.T.reshape(NCHUNK, 128, C)).astype(_BF16)
    bq_h = np.ascontiguousarray(bq.reshape(DQK, 1))
    bk_h = np.ascontiguousarray(bk.reshape(DQK, 1))
    g0 = gamma.reshape(-1)[0]
    gm_h = np.full((128, 1), g0, dtype=_F32)
    gbv = (g0 * bv).astype(_F32)

    in_maps = []
    for core in range(N_CORES):
        b, h = divmod(core, 2)
        in_maps.append(
            {
                "xs": np.ascontiguousarray(xs_full[b]),
                "xt": np.ascontiguousarray(xt_full[b][:, h * NQ : (h + 1) * NQ]),
                "xrt": np.ascontiguousarray(
                    xs_full[b][:, h * NQ : (h + 1) * NQ].T + gbv[None, :]
                ),
                "wq": wq_h,
                "wk": wk_h,
                "wv": wv_h,
                "bq": bq_h,
                "bk": bk_h,
                "gm": gm_h,
            }
        )
    return in_maps


def kernel(x_s, x_t, Wq, bq, Wk, bk, Wv, bv, gamma):
    from concourse.bass_utils import run_bass_kernel_spmd

    in_maps = make_in_maps(x_s, x_t, Wq, bq, Wk, bk, Wv, bv, gamma)
    nc = _get_program()
    res = run_bass_kernel_spmd(nc, in_maps, core_ids=list(range(N_CORES)))

    y = np.empty((B, C, N), dtype=_F32)
    for core in range(N_CORES):
        b, h = divmod(core, 2)
        y[b][:, h * NQ : (h + 1) * NQ] = res.results[core]["outT"].T
    return y.reshape(B, C, W, H)



# revision 2
# speedup vs baseline: 1.0036x; 1.0036x over previous
"""Cross-attention kernel for Trainium2, data-parallel over (batch, query-half)
across 8 NeuronCores. Single-pass bf16 attention with an fp8 DoubleRow V
build, host-precast inputs, and a fused epilogue.

Per-core dataflow:
  - Host pre-casts xs/xt to bf16 (K/Q/E path) and xs to fp8-e4m3 pair layout
    (V path); Wq/Wk bf16, Wv fp8 pairs.
  - V^T built with fp8 DoubleRow matmuls (contracting 256 channels per
    instruction), output to bf16 tiles for the bf16 AV matmuls.
  - Q as 4 group tiles [64, 512], K as 8 block tiles [64, 512]; energy tiles
    [128 j, 512 i] -> exp on the scalar engine (no max subtraction: fp32/bf16
    exp is range-safe for these energies).
  - out^T[i, c] accumulates over j in PSUM; the softmax denominator rides
    along as n=1 matmuls against a ones vector on the same stationary.
  - Epilogue: out = (gamma/s) * av + residual in one fused DVE op; the
    residual is passed pre-transposed with gamma*bv folded in.
"""

import numpy as np
import ml_dtypes

B, C, W, H = 4, 512, 64, 64
N = W * H
DQK = 64
NQ = N // 2
NCHUNK = C // 128
NJ = N // 128
NGROUP = 4
GQ = NQ // NGROUP
NIT = GQ // 128
NBLK = N // 512
N_CORES = 8

_F32 = np.float32
_BF16 = ml_dtypes.bfloat16
_FP8 = ml_dtypes.float8_e4m3fn


def _split_multi_waits(nc, max_waits=1):
    """Peel extra semaphore waits onto NoOps (walrus limit)."""
    from concourse import mybir

    for f in nc.m.functions:
        for bb in f.blocks:
            new_insts = []
            changed = False
            for inst in bb.instructions:
                si = inst.sync_info
                if si is not None and si.on_wait and len(si.on_wait) > max_waits:
                    waits = list(si.on_wait)
                    extra, keep = waits[:-max_waits], waits[-max_waits:]
                    for k in range(0, len(extra), max_waits):
                        nop = mybir.InstNoOp(
                            name=f"{inst.name}-ws{k}",
                            sync_info=mybir.SyncInfo(
                                on_wait=extra[k : k + max_waits], on_update=[]
                            ),
                        )
                        nop.engine = inst.engine
                        new_insts.append(nop)
                    inst.sync_info = mybir.SyncInfo(
                        on_wait=keep, on_update=list(si.on_update)
                    )
                    changed = True
                new_insts.append(inst)
            if changed:
                bb.instructions = new_insts


def build_program():
    import concourse.bass as bass
    import concourse.tile as tile
    from concourse import mybir

    f32 = mybir.dt.float32
    bf16 = mybir.dt.bfloat16
    fp8 = mybir.dt.float8e4
    Alu = mybir.AluOpType
    Act = mybir.ActivationFunctionType
    DR = mybir.MatmulPerfMode.DoubleRow

    nc = bass.Bass("TRN2", target_bir_lowering=False, debug=False, num_devices=1)

    xs = nc.dram_tensor("xs", [C, N], bf16, kind="ExternalInput").ap()
    xsp = nc.dram_tensor("xsp", [2, 128, 2, N], fp8, kind="ExternalInput").ap()
    xt = nc.dram_tensor("xt", [C, NQ], bf16, kind="ExternalInput").ap()
    xres = nc.dram_tensor("xrt", [NQ, C], f32, kind="ExternalInput").ap()
    wq = nc.dram_tensor("wq", [NCHUNK, 128, DQK], bf16, kind="ExternalInput").ap()
    wk = nc.dram_tensor("wk", [NCHUNK, 128, DQK], bf16, kind="ExternalInput").ap()
    wv = nc.dram_tensor("wv", [2, 128, 2, C], fp8, kind="ExternalInput").ap()
    bq = nc.dram_tensor("bq", [DQK, 1], f32, kind="ExternalInput").ap()
    bk = nc.dram_tensor("bk", [DQK, 1], f32, kind="ExternalInput").ap()
    gm = nc.dram_tensor("gm", [128, 1], f32, kind="ExternalInput").ap()
    out = nc.dram_tensor("outT", [NQ, C], f32, kind="ExternalOutput").ap()

    xsv = xs.rearrange("(q p) n -> p q n", p=128)
    xspv = xsp.rearrange("a p b n -> p a b n")
    xtv = xt.rearrange("(q p) n -> p q n", p=128)
    xrv = xres.rearrange("(q p) c -> p q c", p=128)
    outv = out.rearrange("(q p) c -> p q c", p=128)
    wvv = wv.rearrange("a p b c -> p a b c")

    with tile.TileContext(nc) as tc:
        with (
            tc.tile_pool(name="consts", bufs=1) as cpool,
            tc.tile_pool(name="acts", bufs=3) as apool,
            tc.tile_pool(name="qsb", bufs=1) as qpool,
            tc.tile_pool(name="ksb", bufs=1) as kpool,
            tc.tile_pool(name="vtsb", bufs=1) as vpool,
            tc.tile_pool(name="esb", bufs=1) as epool,
            tc.tile_pool(name="small", bufs=4) as spool,
            tc.tile_pool(name="epi", bufs=4) as fpool,
            tc.tile_pool(name="ps_vtp", bufs=2, space="PSUM") as ps_vtp,
            tc.tile_pool(name="ps_kq", bufs=1, space="PSUM") as ps_kq,
            tc.tile_pool(name="ps_e", bufs=2, space="PSUM") as ps_e,
            tc.tile_pool(name="ps_av", bufs=2, space="PSUM") as ps_av,
            tc.tile_pool(name="ps_sum", bufs=1, space="PSUM") as ps_sum,
        ):
            ones = cpool.tile([128, 1], bf16, tag="ones")
            nc.vector.memset(ones[:, :], 1.0)

            wq_sb = cpool.tile([128, NCHUNK, DQK], bf16, tag="wq")
            nc.sync.dma_start(wq_sb[:, :, :], wq.rearrange("q p d -> p q d"))
            wk_sb = cpool.tile([128, NCHUNK, DQK], bf16, tag="wk")
            nc.sync.dma_start(wk_sb[:, :, :], wk.rearrange("q p d -> p q d"))
            wv_sb = cpool.tile([128, 2, 2, C], fp8, tag="wv")
            nc.sync.dma_start(wv_sb[:, :, :, :], wvv)
            bq_sb = cpool.tile([DQK, 1], f32, tag="bq")
            nc.sync.dma_start(bq_sb[:, :], bq[:, :])
            bk_sb = cpool.tile([DQK, 1], f32, tag="bk")
            nc.sync.dma_start(bk_sb[:, :], bk[:, :])
            gm_sb = cpool.tile([128, 1], f32, tag="gm")
            nc.sync.dma_start(gm_sb[:, :], gm[:, :])

            q_g = [
                qpool.tile([DQK, GQ], bf16, tag=f"q{g}", name=f"q{g}")
                for g in range(NGROUP)
            ]
            k_t = [
                kpool.tile([DQK, 512], bf16, tag=f"k{jq}", name=f"k{jq}")
                for jq in range(NBLK)
            ]
            e_tiles = {}

            def emit_e(g, j):
                e_ps = ps_e.tile([128, GQ], f32, tag="eps")
                nc.tensor.matmul(
                    e_ps[:, :],
                    k_t[j // 4][:, (j % 4) * 128 : (j % 4 + 1) * 128],
                    q_g[g][:, :],
                    start=True,
                    stop=True,
                )
                e_j = epool.tile(
                    [128, GQ], bf16, tag=f"e{g % 2}_{j}", name=f"e{g}_{j}"
                )
                nc.scalar.activation(e_j[:, :], e_ps[:, :], Act.Exp)
                e_tiles[(g, j)] = e_j

            vt_t = []
            for jq in range(NBLK):
                bsl = slice(jq * 512, (jq + 1) * 512)
                xsb = apool.tile([128, NCHUNK, 512], bf16, tag="xsb")
                nc.sync.dma_start(xsb[:, :, :], xsv[:, :, bsl])
                xs8 = apool.tile([128, 2, 2, 512], fp8, tag="xs8")
                for cp in range(2):
                    nc.sync.dma_start(xs8[:, cp, :, :], xspv[:, cp, :, bsl])

                for jt in range(4):
                    j = jq * 4 + jt
                    vt_ps = ps_vtp.tile([128, C], f32, tag="vtp")
                    for cp in range(2):
                        nc.tensor.matmul(
                            vt_ps[:, :],
                            xs8[:, cp, :, jt * 128 : (jt + 1) * 128],
                            wv_sb[:, cp, :, :],
                            start=(cp == 0),
                            stop=(cp == 1),
                            perf_mode=DR,
                        )
                    vt_j = vpool.tile([128, C], bf16, tag=f"vt{j}", name=f"vt{j}")
                    nc.vector.tensor_copy(vt_j[:, :], vt_ps[:, :])
                    vt_t.append(vt_j)

                k_ps = ps_kq.tile([DQK, 512], f32, tag="kqp")
                for qc in range(NCHUNK):
                    nc.tensor.matmul(
                        k_ps[:, :],
                        wk_sb[:, qc, :],
                        xsb[:, qc, :],
                        start=(qc == 0),
                        stop=(qc == NCHUNK - 1),
                    )
                nc.vector.tensor_scalar(
                    k_t[jq][:, :], k_ps[:, :], bk_sb[:, :], None, Alu.add
                )

                if jq < NGROUP:
                    g = jq
                    xtb = apool.tile([128, NCHUNK, 512], bf16, tag="xtb")
                    nc.sync.dma_start(
                        xtb[:, :, :], xtv[:, :, g * GQ : (g + 1) * GQ]
                    )
                    q_ps = ps_kq.tile([DQK, 512], f32, tag="kqp")
                    for qc in range(NCHUNK):
                        nc.tensor.matmul(
                            q_ps[:, :],
                            wq_sb[:, qc, :],
                            xtb[:, qc, :],
                            start=(qc == 0),
                            stop=(qc == NCHUNK - 1),
                        )
                    nc.vector.tensor_scalar(
                        q_g[g][:, :], q_ps[:, :], bq_sb[:, :], None, Alu.add
                    )
                if jq >= 1:
                    for j in range(4 * (jq - 1), 4 * jq):
                        emit_e(0, j)

            # ---- attention ----
            for g in range(NGROUP):
                for j in range(NJ):
                    if (g, j) not in e_tiles:
                        emit_e(g, j)
                e_t = [e_tiles[(g, j)] for j in range(NJ)]

                for it in range(NIT):
                    av_ps = ps_av.tile([128, C], f32, tag="av")
                    s_ps = ps_sum.tile([128, 1], f32, tag="sm")
                    isl = slice(it * 128, (it + 1) * 128)
                    for j in range(NJ):
                        nc.tensor.matmul(
                            av_ps[:, :],
                            e_t[j][:, isl],
                            vt_t[j][:, :],
                            start=(j == 0),
                            stop=(j == NJ - 1),
                        )
                        nc.tensor.matmul(
                            s_ps[:, :],
                            e_t[j][:, isl],
                            ones[:, :],
                            start=(j == 0),
                            stop=(j == NJ - 1),
                        )
                    recip = spool.tile([128, 1], f32, tag="rc")
                    nc.vector.reciprocal(recip[:, :], s_ps[:, :])
                    rgam = spool.tile([128, 1], f32, tag="rg")
                    nc.vector.tensor_scalar(
                        rgam[:, :], recip[:, :], gm_sb[:, :], None, Alu.mult
                    )
                    blk = g * NIT + it
                    xr = fpool.tile([128, C], f32, tag="xr")
                    nc.sync.dma_start(xr[:, :], xrv[:, blk, :])
                    of = fpool.tile([128, C], f32, tag="of")
                    for hh in range(2):
                        csl = slice(hh * (C // 2), (hh + 1) * (C // 2))
                        nc.vector.scalar_tensor_tensor(
                            of[:, csl],
                            av_ps[:, csl],
                            rgam[:, :],
                            xr[:, csl],
                            Alu.mult,
                            Alu.add,
                        )
                    nc.sync.dma_start(outv[:, blk, :], of[:, :])
                    if g + 1 < NGROUP:
                        for j in range(8 * it, 8 * it + 8):
                            emit_e(g + 1, j)

    _split_multi_waits(nc)
    return nc


_PROGRAM = None


def _get_program():
    global _PROGRAM
    if _PROGRAM is None:
        _PROGRAM = build_program()
    return _PROGRAM


def make_in_maps(x_s, x_t, Wq, bq, Wk, bk, Wv, bv, gamma):
    x_s = np.asarray(x_s, dtype=_F32)
    x_t = np.asarray(x_t, dtype=_F32)
    Wq = np.asarray(Wq, dtype=_F32)
    Wk = np.asarray(Wk, dtype=_F32)
    Wv = np.asarray(Wv, dtype=_F32)
    bq = np.asarray(bq, dtype=_F32)
    bk = np.asarray(bk, dtype=_F32)
    bv = np.asarray(bv, dtype=_F32)
    gamma = np.asarray(gamma, dtype=_F32)

    xs_full = x_s.reshape(B, C, N)
    xt_full = x_t.reshape(B, C, N)

    wq_h = np.ascontiguousarray(Wq.T.reshape(NCHUNK, 128, DQK)).astype(_BF16)
    wk_h = np.ascontiguousarray(Wk.T.reshape(NCHUNK, 128, DQK)).astype(_BF16)
    wv_c = np.ascontiguousarray(Wv.T.reshape(NCHUNK, 128, C))
    wv_h = np.ascontiguousarray(
        wv_c.reshape(2, 2, 128, C).transpose(0, 2, 1, 3)
    ).astype(_FP8)
    bq_h = np.ascontiguousarray(bq.reshape(DQK, 1))
    bk_h = np.ascontiguousarray(bk.reshape(DQK, 1))
    g0 = gamma.reshape(-1)[0]
    gm_h = np.full((128, 1), g0, dtype=_F32)
    gbv = (g0 * bv).astype(_F32)

    in_maps = []
    for core in range(N_CORES):
        b, h = divmod(core, 2)
        xs_b = xs_full[b]
        xs8 = np.ascontiguousarray(
            xs_b.reshape(2, 2, 128, N).transpose(0, 2, 1, 3)
        ).astype(_FP8)
        in_maps.append(
            {
                "xs": np.ascontiguousarray(xs_b).astype(_BF16),
                "xsp": xs8,
                "xt": np.ascontiguousarray(
                    xt_full[b][:, h * NQ : (h + 1) * NQ]
                ).astype(_BF16),
                "xrt": np.ascontiguousarray(
                    xs_b[:, h * NQ : (h + 1) * NQ].T + gbv[None, :]
                ),
                "wq": wq_h,
                "wk": wk_h,
                "wv": wv_h,
                "bq": bq_h,
                "bk": bk_h,
                "gm": gm_h,
            }
        )
    return in_maps


def kernel(x_s, x_t, Wq, bq, Wk, bk, Wv, bv, gamma):
    from concourse.bass_utils import run_bass_kernel_spmd

    in_maps = make_in_maps(x_s, x_t, Wq, bq, Wk, bk, Wv, bv, gamma)
    nc = _get_program()
    res = run_bass_kernel_spmd(nc, in_maps, core_ids=list(range(N_CORES)))

    y = np.empty((B, C, N), dtype=_F32)
    for core in range(N_CORES):
        b, h = divmod(core, 2)
        y[b][:, h * NQ : (h + 1) * NQ] = res.results[core]["outT"].T
    return y.reshape(B, C, W, H)
